# revision 1
# baseline (speedup 1.0000x reference)
"""Multi-head attention (no softmax) on 8 trn2 NeuronCores.

Reference: out = ((x @ Wqkv.T -> q,k,v per head) ; (q @ k.T * s) @ v ; concat ; @ Wproj.T)

Because there is no softmax the attention is linear:
    (q @ k.T) @ v == q @ (k.T @ v),  k.T @ v is only 64x64 per head,
so the T x T score matrices never need to exist. Per head:
    M_h = (s * k_h).T @ v_h        (64 x 64, reduced over ALL tokens of the batch)
    out += (q_h @ M_h) @ Wproj_h.T

Sharding: token-parallel. Core c owns batch b=c//2, token half c%2 (512 tokens).
M_h needs a reduction over the full batch -> two tiny 128KB AllGathers between
the two cores of each batch (pipelined, peer-add done locally on DVE),
overlapped with the second kv half, the q matmuls, and the first half of the
output projection (which only needs heads 0-7).

All matmuls run in float32r (full PE rate; fp32 is 4x slower). Inputs are
pre-rounded to fp32r on the host (matmul is then exact), intermediates are
rounded by the PSUM->SBUF eviction copies. The head-dim scale 1/8 is folded
into W_k on the host (exact, power of two).

Weights are fed pre-transposed/pre-permuted so every matmul operand has the
contraction dim on partitions with unit-stride DMAs:
  wqkvT (E, 3E): cols 0:E = q features grouped h*64+j, E:2E = k (scaled), 2E:3E = v
  wpT   (E, E):  wpT[f, o] = W_proj[o, f]
  xT_c  (E, 512) per core.

DMA triggers: Sync queue carries x and weights in program order (paces the kv
phase); the GpSimd queue carries the collective bounces so the gathers fire
the moment their inputs are ready; output stores alternate between the two.
"""

import numpy as np

B, T, E = 4, 1024, 1024
NH, HD = 16, 64
N_CORES = 8
TPC = T // 2  # tokens per core = 512

_built = None


def _round_fp32r(a: np.ndarray) -> np.ndarray:
    """Round fp32 to fp32r (11 explicit mantissa bits, RNE) — matches HW."""
    u = np.ascontiguousarray(a, dtype=np.float32).view(np.uint32).astype(np.uint64)
    u = u + 0x7FF + ((u >> 12) & 1)
    u = (u & ~np.uint64(0xFFF)).astype(np.uint32)
    return u.view(np.float32).reshape(a.shape)


def _build():
    """Build + compile the 8-core SPMD Bass program once."""
    global _built
    if _built is not None:
        return _built

    import concourse.mybir as mybir
    import concourse.tile as tile
    from concourse import bacc

    f32 = mybir.dt.float32
    f32r = mybir.dt.float32r
    GROUPS = [[0, 1], [2, 3], [4, 5], [6, 7]]

    nc = bacc.Bacc("TRN2", target_bir_lowering=False, debug=False, num_devices=N_CORES)
    xT = nc.dram_tensor("xT", [E, TPC], f32r, kind="ExternalInput").ap()
    wqkvT = nc.dram_tensor("wqkvT", [E, 3 * E], f32r, kind="ExternalInput").ap()
    wpT = nc.dram_tensor("wpT", [E, E], f32r, kind="ExternalInput").ap()
    out = nc.dram_tensor("out", [TPC, E], f32, kind="ExternalOutput").ap()

    def evict(i, dst, src):
        # spread PSUM->SBUF eviction copies across DVE and ACT
        if i % 2 == 0:
            nc.vector.tensor_copy(dst, src)
        else:
            nc.scalar.copy(dst, src)

    with tile.TileContext(nc) as tc:
        with (
            tc.tile_pool(name="xp", bufs=1) as xp,
            tc.tile_pool(name="wkvp", bufs=4) as wkvp,
            tc.tile_pool(name="kvp", bufs=1) as kvp,
            tc.tile_pool(name="wqp", bufs=2) as wqp,
            tc.tile_pool(name="wpp", bufs=1) as wpp,
            tc.tile_pool(name="qp", bufs=1) as qp,
            tc.tile_pool(name="mres", bufs=1) as mres,
            tc.tile_pool(name="op", bufs=2) as op,
            tc.tile_pool(name="dram", bufs=1, space="DRAM") as dram,
            tc.tile_pool(name="psA", bufs=4, space="PSUM") as psA,
            tc.tile_pool(name="psM", bufs=2, space="PSUM") as psM,
        ):
            # ---- input DMAs ----
            # kv fc-group order: k half 0, v half 0, k half 1, v half 1 so the
            # first half of the M blocks is ready after two groups.
            FC_ORDER = [0, 2, 1, 3]
            xsb = []
            wkv_groups = {}
            for e in range(8):
                t = xp.tile([128, TPC], f32r, tag=f"x{e}")
                nc.sync.dma_start(t[:], xT[128 * e:128 * (e + 1), :])
                xsb.append(t)
                fc = FC_ORDER[0]
                w = wkvp.tile([128, 512], f32r, tag=f"wkv{e}", name=f"wkv{fc}_{e}")
                nc.sync.dma_start(
                    w[:], wqkvT[128 * e:128 * (e + 1), E + 512 * fc:E + 512 * (fc + 1)])
                wkv_groups.setdefault(fc, []).append(w)
            for fc in FC_ORDER[1:]:
                for e in range(8):
                    w = wkvp.tile([128, 512], f32r, tag=f"wkv{e}", name=f"wkv{fc}_{e}")
                    nc.sync.dma_start(
                        w[:], wqkvT[128 * e:128 * (e + 1), E + 512 * fc:E + 512 * (fc + 1)])
                    wkv_groups.setdefault(fc, []).append(w)

            # q weights on the Sync queue AFTER the kv weights so their
            # transfers don't steal HBM bandwidth from the critical kv stream;
            # streamed as two half-column chunk groups
            wq_groups = []
            for h in range(2):
                grp = []
                for e in range(8):
                    t = wqp.tile([128, 512], f32r, tag=f"wq{e}", name=f"wq{h}_{e}")
                    nc.sync.dma_start(
                        t[:], wqkvT[128 * e:128 * (e + 1), 512 * h:512 * (h + 1)])
                    grp.append(t)
                wq_groups.append(grp)

            kvsb = [kvp.tile([128, 2 * E], f32r, tag=f"kv{tt}", name=f"kv{tt}")
                    for tt in range(4)]
            Mbd = mres.tile([128, 1024], f32r, tag="Mbd")
            nc.gpsimd.memset(Mbd[:].bitcast(f32), 0.0)

            bout = [None, None]

            def kv_quarter(fc):
                i = 0
                for tt in range(4):
                    ps = psA.tile([128, 512], f32, tag="big")
                    for e in range(8):
                        nc.tensor.matmul(
                            ps[:],
                            xsb[e][:, 128 * tt:128 * (tt + 1)],
                            wkv_groups[fc][e][:],
                            start=(e == 0), stop=(e == 7),
                        )
                    evict(i, kvsb[tt][:, 512 * fc:512 * (fc + 1)], ps[:])
                    i += 1

            def m_half(g):
                # M blocks 4g..4g+3 from k cols [512g:512g+512], v cols
                # [E+512g : E+512g+512]; keep only diagonal 64x64 sub-blocks.
                mp = psM.tile([128, 512], f32, tag="mp", name=f"mp{g}")
                for j in range(4):
                    blk = 4 * g + j
                    for tt in range(4):
                        nc.tensor.matmul(
                            mp[:, 128 * j:128 * (j + 1)],
                            kvsb[tt][:, 128 * blk:128 * (blk + 1)],
                            kvsb[tt][:, E + 128 * blk:E + 128 * (blk + 1)],
                            start=(tt == 0), stop=(tt == 3),
                        )
                Msb = mres.tile([128, 256], f32, tag=f"Msb{g}", name=f"Msb{g}")
                for j in range(4):
                    nc.vector.tensor_copy(Msb[0:64, 64 * j:64 * j + 64],
                                          mp[0:64, 128 * j:128 * j + 64])
                    nc.vector.tensor_copy(Msb[64:128, 64 * j:64 * j + 64],
                                          mp[64:128, 128 * j + 64:128 * (j + 1)])
                # bounce to DRAM, two DMAs so the transfers ride parallel HW queues
                bin_ = dram.tile([128, 256], f32, name=f"bin{g}")
                bo = dram.tile([256, 256], f32, name=f"bout{g}")
                nc.gpsimd.dma_start(bin_[0:64, :], Msb[0:64, :])
                nc.gpsimd.dma_start(bin_[64:128, :], Msb[64:128, :])
                nc.gpsimd.collective_compute(
                    "AllGather", mybir.AluOpType.bypass, replica_groups=GROUPS,
                    ins=[bin_.opt()], outs=[bo.opt()],
                )
                MrA = mres.tile([128, 256], f32, tag=f"MrA{g}", name=f"MrA{g}")
                MrB = mres.tile([128, 256], f32, tag=f"MrB{g}", name=f"MrB{g}")
                nc.gpsimd.dma_start(MrA[0:64, :], bo[0:64, :])
                nc.gpsimd.dma_start(MrA[64:128, :], bo[64:128, :])
                nc.sync.dma_start(MrB[0:64, :], bo[128:192, :])
                nc.sync.dma_start(MrB[64:128, :], bo[192:256, :])
                bout[g] = (MrA, MrB)

            def m_post(g):
                # add both ranks' partials straight into Mbd diagonal spots
                MrA, MrB = bout[g]
                for j in range(4):
                    blk = 4 * g + j
                    nc.vector.tensor_add(
                        Mbd[0:64, 128 * blk:128 * blk + 64],
                        MrA[0:64, 64 * j:64 * j + 64],
                        MrB[0:64, 64 * j:64 * j + 64])
                    nc.vector.tensor_add(
                        Mbd[64:128, 128 * blk + 64:128 * (blk + 1)],
                        MrA[64:128, 64 * j:64 * j + 64],
                        MrB[64:128, 64 * j:64 * j + 64])

            # ---- kv + M + gathers, pipelined in halves ----
            kv_quarter(0)      # k cols 0:512
            kv_quarter(2)      # v cols 0:512
            m_half(0)          # M blocks 0-3 + AllGather #1 (in flight)
            kv_quarter(1)      # k cols 512:1024
            kv_quarter(3)      # v cols 512:1024
            m_half(1)          # M blocks 4-7 + AllGather #2 (in flight)

            wp = []
            for f in range(8):
                t = wpp.tile([128, E], f32r, tag=f"wp{f}")
                nc.sync.dma_start(t[:], wpT[128 * f:128 * (f + 1), :])
                wp.append(t)

            # ---- q (feature-major qT, (1024f, 512t)), overlaps the gathers ----
            qsb = [qp.tile([128, TPC], f32r, tag=f"q{f}", name=f"q{f}")
                   for f in range(8)]
            for fq in range(8):
                wqg = wq_groups[fq // 4]
                ps = psA.tile([128, 512], f32, tag="big")
                for e in range(8):
                    nc.tensor.matmul(
                        ps[:],
                        wqg[e][:, 128 * (fq % 4):128 * (fq % 4 + 1)],
                        xsb[e][:],
                        start=(e == 0), stop=(e == 7),
                    )
                evict(fq, qsb[fq][:], ps[:])

            m_post(0)
            m_post(1)

            # ---- att: attT_blk = Mbd_blk.T @ qT_blk (in-place into q tiles) ----
            for blk in range(8):
                ps = psA.tile([128, 512], f32, tag="big")
                nc.tensor.matmul(ps[:], Mbd[:, 128 * blk:128 * (blk + 1)],
                                 qsb[blk][:], start=True, stop=True)
                evict(blk, qsb[blk][:], ps[:])
            attsb = qsb

            # ---- out = attT.T @ wpT  ((512t, 1024o)) ----
            i = 0
            for tt in range(4):
                for oc in range(2):
                    ps = psA.tile([128, 512], f32, tag="big")
                    for f in range(8):
                        nc.tensor.matmul(
                            ps[:],
                            attsb[f][:, 128 * tt:128 * (tt + 1)],
                            wp[f][:, 512 * oc:512 * (oc + 1)],
                            start=(f == 0), stop=(f == 7),
                        )
                    ot = op.tile([128, 512], f32, tag="osb")
                    evict(i, ot[:], ps[:])
                    eng = nc.sync if i % 2 else nc.gpsimd
                    i += 1
                    eng.dma_start(
                        out[128 * tt:128 * (tt + 1), 512 * oc:512 * (oc + 1)],
                        ot[:],
                    )

    nc.compile()
    _built = nc
    return nc


LAST_RESULTS = None  # BassKernelResults of the most recent kernel() call


def kernel(x: np.ndarray, W_qkv: np.ndarray, W_proj: np.ndarray) -> np.ndarray:
    global LAST_RESULTS
    from concourse import bass_utils

    nc = _build()

    x = np.ascontiguousarray(x, dtype=np.float32)
    W_qkv = np.ascontiguousarray(W_qkv, dtype=np.float32)
    W_proj = np.ascontiguousarray(W_proj, dtype=np.float32)

    # head-grouping permutation: grouped feature h*64+j <- original row j*16+h
    perm = np.arange(E).reshape(HD, NH).T.ravel()
    Wq_g = W_qkv[perm]
    Wk_g = W_qkv[E + perm] * np.float32(HD ** -0.5)  # exact: 1/8
    Wv_g = W_qkv[2 * E + perm]
    wqkvT_np = _round_fp32r(np.concatenate([Wq_g, Wk_g, Wv_g], 0).T)
    wpT_np = _round_fp32r(W_proj.T)

    in_maps = []
    for c in range(N_CORES):
        b, half = c // 2, c % 2
        xT_c = _round_fp32r(x[b, half * TPC:(half + 1) * TPC, :].T)
        in_maps.append({"xT": xT_c, "wqkvT": wqkvT_np, "wpT": wpT_np})

    import os as _os
    _tc = _os.environ.get("KERNEL_TRACE_CORES")
    _kw = {"trace_cores": [int(x) for x in _tc.split(",")]} if _tc else {}
    res = bass_utils.run_bass_kernel_spmd(nc, in_maps, core_ids=list(range(N_CORES)), **_kw)
    LAST_RESULTS = res

    out = np.empty((B, T, E), dtype=np.float32)
    for c in range(N_CORES):
        b, half = c // 2, c % 2
        out[b, half * TPC:(half + 1) * TPC, :] = res.results[c]["out"]
    return out



# revision 3
# speedup vs baseline: 1.0258x; 1.0258x over previous
"""Multi-head attention (no softmax) on 8 trn2 NeuronCores.

Reference: out = ((x @ Wqkv.T -> q,k,v per head) ; (q @ k.T * s) @ v ; concat ; @ Wproj.T)

Because there is no softmax the attention is linear:
    (q @ k.T) @ v == q @ (k.T @ v),  k.T @ v is only 64x64 per head,
so the T x T score matrices never need to exist. Per head:
    M_h = (s * k_h).T @ v_h        (64 x 64, reduced over ALL tokens of the batch)
    out += (q_h @ M_h) @ Wproj_h.T

Sharding: token-parallel. Core c owns batch b=c//2, token half c%2 (512 tokens).
M_h needs a reduction over the full batch -> two tiny 64KB AllReduce(add)s
between the two cores of each batch, overlapped with the q matmuls.

Everything runs in bf16 (same PE rate as fp32r, half the HBM traffic, and
128-row matmuls stay full rate; end-to-end rel err ~5e-3 vs the 2e-2 gate).
PSUM accumulation is fp32. The head-dim scale 1/8 is folded into W_k on the
host (exact, power of two).

DMAs are coarse: the host pre-swizzles every weight group into the exact
[128, cols] SBUF layout so each group is ONE contiguous dma_start (the Sync
sequencer spends ~0.6us of issue time PER dma_start; the f32 baseline burned
~40us there on 64 transfers). Load order is by first use: wkv group k0, the
8 x tiles (so kv matmuls start as x trickles in), then v0,k1,v1, wq, wp.

Latency hiders: 8 dummy matmuls at t=0 ramp the PE out of its low p-state
while the first DMAs land; a dummy 256B AllReduce at t=0 warms the CC core
(cold-start costs ~11us, warm ~1.2us). AllReduce results DMA straight into
the block-diagonal M tile (zeroed once), no local peer-add needed.
"""

import numpy as np

B, T, E = 4, 1024, 1024
NH, HD = 16, 64
N_CORES = 8
TPC = T // 2  # tokens per core = 512

_built = None


def _build():
    """Build + compile the 8-core SPMD Bass program once."""
    global _built
    if _built is not None:
        return _built

    import concourse.mybir as mybir
    import concourse.tile as tile
    from concourse import bacc

    f32 = mybir.dt.float32
    bf16 = mybir.dt.bfloat16
    GROUPS = [[0, 1], [2, 3], [4, 5], [6, 7]]

    nc = bacc.Bacc("TRN2", target_bir_lowering=False, debug=False, num_devices=N_CORES)
    # x: [1024 xfeat, 512 tok]; row-block e -> sbuf tile [128, 512]
    xd = nc.dram_tensor("xd", [E, TPC], bf16, kind="ExternalInput").ap()
    # kv weights: 4 stream groups (k0, v0, k1, v1), each [128 part, e*512 + kvf]
    wkvd = nc.dram_tensor("wkvd", [4 * 128, 8 * 512], bf16, kind="ExternalInput").ap()
    # q weights: 2 column-half groups, each [128 part, e*512 + qf]
    wqd = nc.dram_tensor("wqd", [2 * 128, 8 * 512], bf16, kind="ExternalInput").ap()
    # proj weights: [128 part, f*1024 + o]
    wpd = nc.dram_tensor("wpd", [128, 8 * 1024], bf16, kind="ExternalInput").ap()
    out = nc.dram_tensor("out", [TPC, E], f32, kind="ExternalOutput").ap()

    evict_i = [0]

    def evict(dst, src):
        # spread PSUM->SBUF eviction copies across DVE and ACT
        if evict_i[0] % 2 == 0:
            nc.vector.tensor_copy(dst, src)
        else:
            nc.scalar.copy(dst, src)
        evict_i[0] += 1

    with tile.TileContext(nc) as tc:
        with (
            tc.tile_pool(name="xp", bufs=1) as xp,
            tc.tile_pool(name="wkvp", bufs=1) as wkvp,
            tc.tile_pool(name="kvp", bufs=1) as kvp,
            tc.tile_pool(name="wqp", bufs=1) as wqp,
            tc.tile_pool(name="wpp", bufs=1) as wpp,
            tc.tile_pool(name="qp", bufs=1) as qp,
            tc.tile_pool(name="mres", bufs=1) as mres,
            tc.tile_pool(name="op", bufs=2) as op,
            tc.tile_pool(name="warm", bufs=1) as warmp,
            tc.tile_pool(name="dram", bufs=1, space="DRAM") as dram,
            tc.tile_pool(name="psA", bufs=5, space="PSUM") as psA,
            tc.tile_pool(name="psM", bufs=2, space="PSUM") as psM,
            tc.tile_pool(name="psW", bufs=1, space="PSUM") as psW,
        ):
            # ---- t=0: warm the PE (p-state ramp) and the CC core ----
            warm = warmp.tile([128, 512], bf16, tag="warm")
            nc.gpsimd.memset(warm[:].bitcast(f32), 0.0)
            wsrc = warmp.tile([1, 64], f32, tag="wsrc")
            nc.gpsimd.memset(wsrc[:], 0.0)
            Mbd = mres.tile([128, 1024], bf16, tag="Mbd")
            nc.gpsimd.memset(Mbd[:].bitcast(f32), 0.0)

            wbin = dram.tile([1, 64], f32, name="wbin")
            wbo = dram.tile([1, 64], f32, name="wbo")
            nc.gpsimd.dma_start(wbin[:], wsrc[:])
            nc.gpsimd.collective_compute(
                "AllReduce", mybir.AluOpType.add, replica_groups=GROUPS,
                ins=[wbin.opt()], outs=[wbo.opt()],
            )

            psw = psW.tile([128, 512], f32, tag="psw")
            for _ in range(8):
                nc.tensor.matmul(psw[:], warm[:, 0:128], warm[:],
                                 start=True, stop=True)

            # ---- input DMAs (one per logical group, ordered by first use) ----
            xsb = [xp.tile([128, TPC], bf16, tag=f"x{e}", name=f"x{e}")
                   for e in range(8)]
            KV_SLOT = [0, 2, 1, 3]  # stream order k0, v0, k1, v1 -> kvsb col slot
            wkv = [wkvp.tile([128, 4096], bf16, tag=f"wkv{s}", name=f"wkv{s}")
                   for s in range(4)]
            wq = [wqp.tile([128, 4096], bf16, tag=f"wq{h}", name=f"wq{h}")
                  for h in range(2)]
            wp = wpp.tile([128, 8192], bf16, tag="wp")

            nc.sync.dma_start(wkv[0][:], wkvd[0:128, :])
            for e in range(8):
                nc.sync.dma_start(xsb[e][:], xd[128 * e:128 * (e + 1), :])
            for s in range(1, 4):
                nc.sync.dma_start(wkv[s][:], wkvd[128 * s:128 * (s + 1), :])
            for h in range(2):
                nc.sync.dma_start(wq[h][:], wqd[128 * h:128 * (h + 1), :])
            nc.sync.dma_start(wp[:], wpd[:, :])

            # kvsb[tt]: [128 tok, 2048] cols = [k(1024) | v(1024)] grouped feats
            kvsb = [kvp.tile([128, 2048], bf16, tag=f"kv{tt}", name=f"kv{tt}")
                    for tt in range(4)]

            def kv_quarter(s):
                slot = KV_SLOT[s]
                for tt in range(4):
                    ps = psA.tile([128, 512], f32, tag="big")
                    for e in range(8):
                        nc.tensor.matmul(
                            ps[:],
                            xsb[e][:, 128 * tt:128 * (tt + 1)],
                            wkv[s][:, 512 * e:512 * (e + 1)],
                            start=(e == 0), stop=(e == 7),
                        )
                    evict(kvsb[tt][:, 512 * slot:512 * (slot + 1)], ps[:])

            def m_half(g):
                # M blocks 4g..4g+3 (2 heads per 128-block, diagonal 64x64s),
                # AllReduce(add) over the batch's core pair
                mp = psM.tile([128, 512], f32, tag="mp", name=f"mp{g}")
                for j in range(4):
                    blk = 4 * g + j
                    for tt in range(4):
                        nc.tensor.matmul(
                            mp[:, 128 * j:128 * (j + 1)],
                            kvsb[tt][:, 128 * blk:128 * (blk + 1)],
                            kvsb[tt][:, 1024 + 128 * blk:1024 + 128 * (blk + 1)],
                            start=(tt == 0), stop=(tt == 3),
                        )
                # extract the 8 diagonal 64x64 blocks -> Msb [128, 256] bf16
                Msb = mres.tile([128, 256], bf16, tag=f"Msb{g}", name=f"Msb{g}")
                mpv = mp[:].rearrange("p (j c) -> p j c", j=4)
                msv = Msb[:].rearrange("p (j c) -> p j c", j=4)
                nc.vector.tensor_copy(msv[0:64], mpv[0:64, :, 0:64])
                nc.scalar.copy(msv[64:128], mpv[64:128, :, 64:128])
                bin_ = dram.tile([128, 256], bf16, name=f"bin{g}")
                bo = dram.tile([128, 256], bf16, name=f"bo{g}")
                nc.gpsimd.dma_start(bin_[:], Msb[:])
                nc.gpsimd.collective_compute(
                    "AllReduce", mybir.AluOpType.add, replica_groups=GROUPS,
                    ins=[bin_.opt()], outs=[bo.opt()],
                )
                # summed result lands straight in Mbd's diagonal spots
                mbv = Mbd[:, 512 * g:512 * (g + 1)].rearrange("p (j c) -> p j c", j=4)
                bov = bo[:].rearrange("p (j c) -> p j c", j=4)
                nc.sync.dma_start(mbv[0:64, :, 0:64], bov[0:64])
                nc.sync.dma_start(mbv[64:128, :, 64:128], bov[64:128])

            # ---- kv + M + allreduces, pipelined in halves ----
            kv_quarter(0)      # k0
            kv_quarter(1)      # v0
            m_half(0)
            kv_quarter(2)      # k1
            kv_quarter(3)      # v1
            m_half(1)

            # ---- q (feature-major, [128 qf, 512 tok] per block), overlaps CC ----
            qsb = [qp.tile([128, TPC], bf16, tag=f"q{f}", name=f"q{f}")
                   for f in range(8)]
            for fq in range(8):
                wqh = wq[fq // 4]
                l = fq % 4
                ps = psA.tile([128, 512], f32, tag="big")
                for e in range(8):
                    nc.tensor.matmul(
                        ps[:],
                        wqh[:, 512 * e + 128 * l:512 * e + 128 * (l + 1)],
                        xsb[e][:],
                        start=(e == 0), stop=(e == 7),
                    )
                evict(qsb[fq][:], ps[:])

            # ---- att: attT_blk = Mbd_blk.T @ qT_blk (in-place into q tiles) ----
            for blk in range(8):
                ps = psA.tile([128, 512], f32, tag="big")
                nc.tensor.matmul(ps[:], Mbd[:, 128 * blk:128 * (blk + 1)],
                                 qsb[blk][:], start=True, stop=True)
                evict(qsb[blk][:], ps[:])
            attsb = qsb

            # ---- out = attT.T @ wp  ([512 tok, 1024 o]) ----
            for tt in range(4):
                for oc in range(2):
                    ps = psA.tile([128, 512], f32, tag="big")
                    for f in range(8):
                        nc.tensor.matmul(
                            ps[:],
                            attsb[f][:, 128 * tt:128 * (tt + 1)],
                            wp[:, 1024 * f + 512 * oc:1024 * f + 512 * (oc + 1)],
                            start=(f == 0), stop=(f == 7),
                        )
                    ot = op.tile([128, 512], f32, tag="osb")
                    evict(ot[:], ps[:])
                    nc.sync.dma_start(
                        out[128 * tt:128 * (tt + 1), 512 * oc:512 * (oc + 1)],
                        ot[:],
                    )

    nc.compile()
    _built = nc
    return nc


LAST_RESULTS = None  # BassKernelResults of the most recent kernel() call


def _swz(a: np.ndarray) -> np.ndarray:
    """[1024, C] -> [128, 8*C]: row e*128+p -> partition p, cols e*C..e*C+C."""
    C = a.shape[1]
    return np.ascontiguousarray(
        a.reshape(8, 128, C).transpose(1, 0, 2).reshape(128, 8 * C))


def kernel(x: np.ndarray, W_qkv: np.ndarray, W_proj: np.ndarray) -> np.ndarray:
    global LAST_RESULTS
    from ml_dtypes import bfloat16
    from concourse import bass_utils

    nc = _build()

    x = np.ascontiguousarray(x, dtype=np.float32)
    W_qkv = np.ascontiguousarray(W_qkv, dtype=np.float32)
    W_proj = np.ascontiguousarray(W_proj, dtype=np.float32)

    # head-grouping permutation: grouped feature h*64+j <- original row j*16+h
    perm = np.arange(E).reshape(HD, NH).T.ravel()
    Wq_g = W_qkv[perm].astype(bfloat16)
    Wk_g = (W_qkv[E + perm] * np.float32(HD ** -0.5)).astype(bfloat16)  # exact 1/8
    Wv_g = W_qkv[2 * E + perm].astype(bfloat16)
    Wp_g = W_proj.astype(bfloat16)  # att concat order == grouped order already

    # kv stream groups k0, v0, k1, v1: each [512 kvf, 1024 xf] -> swz([1024, 512])
    kv_groups = [Wk_g[0:512], Wv_g[0:512], Wk_g[512:1024], Wv_g[512:1024]]
    wkvd_np = np.concatenate([_swz(np.ascontiguousarray(g.T)) for g in kv_groups], 0)
    wqd_np = np.concatenate(
        [_swz(np.ascontiguousarray(Wq_g[512 * h:512 * (h + 1)].T)) for h in range(2)], 0)
    wpd_np = _swz(np.ascontiguousarray(Wp_g.T))

    in_maps = []
    for c in range(N_CORES):
        b, half = c // 2, c % 2
        xd_c = np.ascontiguousarray(
            x[b, half * TPC:(half + 1) * TPC, :].T.astype(bfloat16))
        in_maps.append({"xd": xd_c, "wkvd": wkvd_np, "wqd": wqd_np, "wpd": wpd_np})

    import os as _os
    _tc = _os.environ.get("KERNEL_TRACE_CORES")
    _kw = {"trace_cores": [int(v) for v in _tc.split(",")]} if _tc else {}
    res = bass_utils.run_bass_kernel_spmd(nc, in_maps, core_ids=list(range(N_CORES)), **_kw)
    LAST_RESULTS = res

    out = np.empty((B, T, E), dtype=np.float32)
    for c in range(N_CORES):
        b, half = c // 2, c % 2
        out[b, half * TPC:(half + 1) * TPC, :] = res.results[c]["out"]
    return out


# revision 4
# speedup vs baseline: 1.0653x; 1.0385x over previous
"""Multi-head attention (no softmax) on 8 trn2 NeuronCores.

Reference: out = ((x @ Wqkv.T -> q,k,v per head) ; (q @ k.T * s) @ v ; concat ; @ Wproj.T)

Because there is no softmax the attention is linear:
    (q @ k.T) @ v == q @ (k.T @ v),  k.T @ v is only 64x64 per head,
so the T x T score matrices never need to exist. Per head:
    M_h = (s * k_h).T @ v_h        (64 x 64, reduced over ALL tokens of the batch)
    out += (q_h @ M_h) @ Wproj_h.T

Sharding: token-parallel. Core c owns batch b=c//2, token half c%2 (512 tokens).
M_h needs a reduction over the full batch -> two tiny 64KB AllReduce(add)s
between the two cores of each batch.

Everything runs in bf16 (same PE rate as fp32r, half the HBM traffic; rel err
~5e-3 vs the 2e-2 gate). PSUM accumulates fp32. The 1/8 head scale is folded
into W_k on the host (exact).

Collective physics on this platform (measured): each collective op is a
global 8-core mesh; the FIRST op prepends ~12us of semaphore hops that make
no progress while the DMA engines are saturated; chained ops start ~1-2us
after the previous. So: the pre-collective bulk is capped at 6MB (wkv+x+wq,
drains ~31us, right when AllReduce#1 triggers), W_proj is deferred to the
scalar queue in two 1MB halves, and the output projection is split so that
~7us of AR2-independent matmuls (att blocks 0-3 -> out partials f=0..3 with
PSUM groups held open) cover AR2's latency.

DMAs are coarse - the host pre-swizzles each weight group into its exact
[128, cols] SBUF layout so every logical tensor is ONE contiguous dma_start
(the Sync sequencer spends ~0.6us of issue time per dma_start; the f32
baseline burned ~40us there on 64 transfers). 12 dummy matmuls at t=0 ramp
the PE out of its low p-state while the first DMAs land.
"""

import numpy as np

B, T, E = 4, 1024, 1024
NH, HD = 16, 64
N_CORES = 8
TPC = T // 2  # tokens per core = 512

_built = None


def _build():
    """Build + compile the 8-core SPMD Bass program once."""
    global _built
    if _built is not None:
        return _built

    import concourse.mybir as mybir
    import concourse.tile as tile
    from concourse import bacc

    f32 = mybir.dt.float32
    bf16 = mybir.dt.bfloat16
    GROUPS = [[0, 1], [2, 3], [4, 5], [6, 7]]

    nc = bacc.Bacc("TRN2", target_bir_lowering=False, debug=False, num_devices=N_CORES)
    # x: [1024 xfeat, 512 tok]; row-block e -> sbuf tile [128, 512]
    xd = nc.dram_tensor("xd", [E, TPC], bf16, kind="ExternalInput").ap()
    # kv weights: 4 stream groups (k0, v0, k1, v1), each [128 part, e*512 + kvf]
    wkvd = nc.dram_tensor("wkvd", [4 * 128, 8 * 512], bf16, kind="ExternalInput").ap()
    # q weights: 2 column-half groups, each [128 part, e*512 + qf]
    wqd = nc.dram_tensor("wqd", [2 * 128, 8 * 512], bf16, kind="ExternalInput").ap()
    # proj weights, oc-major: [128 part, oc*4096 + f*512 + c]
    wpd = nc.dram_tensor("wpd", [128, 8 * 1024], bf16, kind="ExternalInput").ap()
    out = nc.dram_tensor("out", [TPC, E], f32, kind="ExternalOutput").ap()

    evict_i = [0]

    def evict(dst, src):
        # spread PSUM->SBUF eviction copies across DVE and ACT
        if evict_i[0] % 2 == 0:
            nc.vector.tensor_copy(dst, src)
        else:
            nc.scalar.copy(dst, src)
        evict_i[0] += 1

    with tile.TileContext(nc) as tc:
        with (
            tc.tile_pool(name="xp", bufs=1) as xp,
            tc.tile_pool(name="wkvp", bufs=1) as wkvp,
            tc.tile_pool(name="kvp", bufs=1) as kvp,
            tc.tile_pool(name="wqp", bufs=1) as wqp,
            tc.tile_pool(name="wpp", bufs=1) as wpp,
            tc.tile_pool(name="qp", bufs=1) as qp,
            tc.tile_pool(name="mres", bufs=1) as mres,
            tc.tile_pool(name="op", bufs=2) as op,
            tc.tile_pool(name="warm", bufs=1) as warmp,
            tc.tile_pool(name="dram", bufs=1, space="DRAM") as dram,
            tc.tile_pool(name="psA", bufs=5, space="PSUM") as psA,
            tc.tile_pool(name="psM", bufs=2, space="PSUM") as psM,
            tc.tile_pool(name="psW", bufs=1, space="PSUM") as psW,
        ):
            # ---- t=0: PE p-state warmup while the first DMAs land ----
            warm = warmp.tile([128, 512], bf16, tag="warm")
            nc.gpsimd.memset(warm[:].bitcast(f32), 0.0)
            Mbd = mres.tile([128, 1024], bf16, tag="Mbd")
            nc.gpsimd.memset(Mbd[:].bitcast(f32), 0.0)
            psw = psW.tile([128, 512], f32, tag="psw")
            for _ in range(12):
                nc.tensor.matmul(psw[:], warm[:, 0:128], warm[:],
                                 start=True, stop=True)

            # ---- phase-1 input DMAs (6MB; wp deferred to the scalar queue) ----
            xsb = [xp.tile([128, TPC], bf16, tag=f"x{e}", name=f"x{e}")
                   for e in range(8)]
            KV_SLOT = [0, 2, 1, 3]  # stream order k0, v0, k1, v1 -> kvsb col slot
            wkv = [wkvp.tile([128, 4096], bf16, tag=f"wkv{s}", name=f"wkv{s}")
                   for s in range(4)]
            wq = [wqp.tile([128, 4096], bf16, tag=f"wq{h}", name=f"wq{h}")
                  for h in range(2)]
            wp = wpp.tile([128, 8192], bf16, tag="wp")

            nc.sync.dma_start(wkv[0][:], wkvd[0:128, :])
            for e in range(8):
                nc.sync.dma_start(xsb[e][:], xd[128 * e:128 * (e + 1), :])
            for s in range(1, 4):
                nc.sync.dma_start(wkv[s][:], wkvd[128 * s:128 * (s + 1), :])
            for h in range(2):
                nc.sync.dma_start(wq[h][:], wqd[128 * h:128 * (h + 1), :])

            # kvsb[tt]: [128 tok, 2048] cols = [k(1024) | v(1024)] grouped feats
            kvsb = [kvp.tile([128, 2048], bf16, tag=f"kv{tt}", name=f"kv{tt}")
                    for tt in range(4)]

            def kv_quarter(s):
                slot = KV_SLOT[s]
                for tt in range(4):
                    ps = psA.tile([128, 512], f32, tag="big")
                    for e in range(8):
                        nc.tensor.matmul(
                            ps[:],
                            xsb[e][:, 128 * tt:128 * (tt + 1)],
                            wkv[s][:, 512 * e:512 * (e + 1)],
                            start=(e == 0), stop=(e == 7),
                        )
                    evict(kvsb[tt][:, 512 * slot:512 * (slot + 1)], ps[:])

            def m_half(g):
                # M blocks 4g..4g+3 (2 heads per 128-block, diagonal 64x64s),
                # AllReduce(add) over the batch's core pair
                mp = psM.tile([128, 512], f32, tag="mp", name=f"mp{g}")
                for j in range(4):
                    blk = 4 * g + j
                    for tt in range(4):
                        nc.tensor.matmul(
                            mp[:, 128 * j:128 * (j + 1)],
                            kvsb[tt][:, 128 * blk:128 * (blk + 1)],
                            kvsb[tt][:, 1024 + 128 * blk:1024 + 128 * (blk + 1)],
                            start=(tt == 0), stop=(tt == 3),
                        )
                # extract the 8 diagonal 64x64 blocks -> Msb [128, 256] bf16
                Msb = mres.tile([128, 256], bf16, tag=f"Msb{g}", name=f"Msb{g}")
                mpv = mp[:].rearrange("p (j c) -> p j c", j=4)
                msv = Msb[:].rearrange("p (j c) -> p j c", j=4)
                nc.vector.tensor_copy(msv[0:64], mpv[0:64, :, 0:64])
                nc.scalar.copy(msv[64:128], mpv[64:128, :, 64:128])
                # deferred wp half rides the scalar queue here, during the
                # collective's op window (issue only, doesn't block the queue)
                nc.scalar.dma_start(wp[:, 4096 * g:4096 * (g + 1)],
                                    wpd[:, 4096 * g:4096 * (g + 1)])
                bin_ = dram.tile([128, 256], bf16, name=f"bin{g}")
                bo = dram.tile([128, 256], bf16, name=f"bo{g}")
                nc.gpsimd.dma_start(bin_[:], Msb[:])
                nc.gpsimd.collective_compute(
                    "AllReduce", mybir.AluOpType.add, replica_groups=GROUPS,
                    ins=[bin_.opt()], outs=[bo.opt()],
                )
                # summed result lands straight in Mbd's diagonal spots
                mbv = Mbd[:, 512 * g:512 * (g + 1)].rearrange("p (j c) -> p j c", j=4)
                bov = bo[:].rearrange("p (j c) -> p j c", j=4)
                nc.sync.dma_start(mbv[0:64, :, 0:64], bov[0:64])
                nc.sync.dma_start(mbv[64:128, :, 64:128], bov[64:128])

            # ---- kv + M + allreduces, pipelined in halves ----
            kv_quarter(0)      # k0
            kv_quarter(1)      # v0
            m_half(0)
            kv_quarter(2)      # k1
            kv_quarter(3)      # v1
            m_half(1)

            # ---- q (feature-major, [128 qf, 512 tok] per block), overlaps CC ----
            qsb = [qp.tile([128, TPC], bf16, tag=f"q{f}", name=f"q{f}")
                   for f in range(8)]
            for fq in range(8):
                wqh = wq[fq // 4]
                l = fq % 4
                ps = psA.tile([128, 512], f32, tag="big")
                for e in range(8):
                    nc.tensor.matmul(
                        ps[:],
                        wqh[:, 512 * e + 128 * l:512 * e + 128 * (l + 1)],
                        xsb[e][:],
                        start=(e == 0), stop=(e == 7),
                    )
                evict(qsb[fq][:], ps[:])

            # ---- att blk = Mbd_blk.T @ q_blk, in-place into the q tiles.
            # Blocks 0-3 need only AllReduce#1; 4-7 wait on AllReduce#2, so
            # the out projection is interleaved to hide AR2's latency. ----
            def att(blk):
                ps = psM.tile([128, 512], f32, tag="mp", name=f"att{blk}")
                nc.tensor.matmul(ps[:], Mbd[:, 128 * blk:128 * (blk + 1)],
                                 qsb[blk][:], start=True, stop=True)
                evict(qsb[blk][:], ps[:])
            attsb = qsb

            for blk in range(4):
                att(blk)

            # out oc=0 columns: open 4 PSUM groups with the f=0..3 partials
            # (only att blocks 0-3 needed), finish f=4..7 after AR2 lands
            ps_oc0 = []
            for tt in range(4):
                ps = psA.tile([128, 512], f32, tag="big", name=f"out0_{tt}")
                for f in range(4):
                    nc.tensor.matmul(
                        ps[:],
                        attsb[f][:, 128 * tt:128 * (tt + 1)],
                        wp[:, 512 * f:512 * (f + 1)],
                        start=(f == 0), stop=False,
                    )
                ps_oc0.append(ps)

            for blk in range(4, 8):
                att(blk)

            for tt in range(4):
                ps = ps_oc0[tt]
                for f in range(4, 8):
                    nc.tensor.matmul(
                        ps[:],
                        attsb[f][:, 128 * tt:128 * (tt + 1)],
                        wp[:, 512 * f:512 * (f + 1)],
                        start=False, stop=(f == 7),
                    )
                ot = op.tile([128, 512], f32, tag="osb")
                evict(ot[:], ps[:])
                nc.sync.dma_start(out[128 * tt:128 * (tt + 1), 0:512], ot[:])

            # out oc=1 columns: everything available, straight accumulation
            for tt in range(4):
                ps = psA.tile([128, 512], f32, tag="big", name=f"out1_{tt}")
                for f in range(8):
                    nc.tensor.matmul(
                        ps[:],
                        attsb[f][:, 128 * tt:128 * (tt + 1)],
                        wp[:, 4096 + 512 * f:4096 + 512 * (f + 1)],
                        start=(f == 0), stop=(f == 7),
                    )
                ot = op.tile([128, 512], f32, tag="osb")
                evict(ot[:], ps[:])
                nc.sync.dma_start(out[128 * tt:128 * (tt + 1), 512:1024], ot[:])

    nc.compile()
    _built = nc
    return nc


LAST_RESULTS = None  # BassKernelResults of the most recent kernel() call


def _swz(a: np.ndarray) -> np.ndarray:
    """[1024, C] -> [128, 8*C]: row e*128+p -> partition p, cols e*C..e*C+C."""
    C = a.shape[1]
    return np.ascontiguousarray(
        a.reshape(8, 128, C).transpose(1, 0, 2).reshape(128, 8 * C))


def kernel(x: np.ndarray, W_qkv: np.ndarray, W_proj: np.ndarray) -> np.ndarray:
    global LAST_RESULTS
    from ml_dtypes import bfloat16
    from concourse import bass_utils

    nc = _build()

    x = np.ascontiguousarray(x, dtype=np.float32)
    W_qkv = np.ascontiguousarray(W_qkv, dtype=np.float32)
    W_proj = np.ascontiguousarray(W_proj, dtype=np.float32)

    # head-grouping permutation: grouped feature h*64+j <- original row j*16+h
    perm = np.arange(E).reshape(HD, NH).T.ravel()
    Wq_g = W_qkv[perm].astype(bfloat16)
    Wk_g = (W_qkv[E + perm] * np.float32(HD ** -0.5)).astype(bfloat16)  # exact 1/8
    Wv_g = W_qkv[2 * E + perm].astype(bfloat16)
    Wp_g = W_proj.astype(bfloat16)  # att concat order == grouped order already

    # kv stream groups k0, v0, k1, v1: each [512 kvf, 1024 xf] -> swz([1024, 512])
    kv_groups = [Wk_g[0:512], Wv_g[0:512], Wk_g[512:1024], Wv_g[512:1024]]
    wkvd_np = np.concatenate([_swz(np.ascontiguousarray(g.T)) for g in kv_groups], 0)
    wqd_np = np.concatenate(
        [_swz(np.ascontiguousarray(Wq_g[512 * h:512 * (h + 1)].T)) for h in range(2)], 0)
    # wp oc-major: [128 p(af in f), oc*4096 + f*512 + c], wp[p, ...] = Wp[o, af]
    wpT = np.ascontiguousarray(Wp_g.T)           # [1024 af, 1024 o]
    w = wpT.reshape(8, 128, 2, 512)              # [f, p, oc, c]
    wpd_np = np.ascontiguousarray(
        w.transpose(1, 2, 0, 3).reshape(128, 8192))  # [p, oc, f, c]

    in_maps = []
    for c in range(N_CORES):
        b, half = c // 2, c % 2
        xd_c = np.ascontiguousarray(
            x[b, half * TPC:(half + 1) * TPC, :].T.astype(bfloat16))
        in_maps.append({"xd": xd_c, "wkvd": wkvd_np, "wqd": wqd_np, "wpd": wpd_np})

    import os as _os
    _tc = _os.environ.get("KERNEL_TRACE_CORES")
    _kw = {"trace_cores": [int(v) for v in _tc.split(",")]} if _tc else {}
    res = bass_utils.run_bass_kernel_spmd(nc, in_maps, core_ids=list(range(N_CORES)), **_kw)
    LAST_RESULTS = res

    out = np.empty((B, T, E), dtype=np.float32)
    for c in range(N_CORES):
        b, half = c // 2, c % 2
        out[b, half * TPC:(half + 1) * TPC, :] = res.results[c]["out"]
    return out


# revision 10
# speedup vs baseline: 1.0810x; 1.0147x over previous
"""Multi-head attention (no softmax) on 8 trn2 NeuronCores.

Reference: out = ((x @ Wqkv.T -> q,k,v per head) ; (q @ k.T * s) @ v ; concat ; @ Wproj.T)

Because there is no softmax the attention is linear:
    (q @ k.T) @ v == q @ (k.T @ v),  k.T @ v is only 64x64 per head,
so the T x T score matrices never need to exist. Per head:
    M_h = (s * k_h).T @ v_h        (64 x 64, reduced over ALL tokens of the batch)
    out += (q_h @ M_h) @ Wproj_h.T

Sharding: token-parallel. Core c owns batch b=c//2, token half c%2 (512 tokens).
M_h needs a reduction over the full batch -> two tiny 64KB AllReduce(add)s
between the two cores of each batch.

Everything runs in bf16 (same PE rate as fp32r, half the HBM traffic; rel err
~5e-3 vs the 2e-2 gate). PSUM accumulates fp32. The 1/8 head scale is folded
into W_k on the host (exact).

Collective physics on this platform (measured): each collective op is a
global 8-core mesh; the FIRST op prepends ~12us of semaphore hops that make
no progress while the DMA engines are saturated; chained ops start ~1-2us
after the previous. So: the pre-collective bulk is capped at 6MB (wkv+x+wq,
drains ~31us, right when AllReduce#1 triggers), W_proj is deferred to the
scalar queue in two 1MB halves, and the output projection is split so that
~7us of AR2-independent matmuls (att blocks 0-3 -> out partials f=0..3 with
PSUM groups held open) cover AR2's latency.

DMAs are coarse - the host pre-swizzles each weight group into its exact
[128, cols] SBUF layout so every logical tensor is ONE contiguous dma_start
(the Sync sequencer spends ~0.6us of issue time per dma_start; the f32
baseline burned ~40us there on 64 transfers). 12 dummy matmuls at t=0 ramp
the PE out of its low p-state while the first DMAs land.
"""

import numpy as np

B, T, E = 4, 1024, 1024
NH, HD = 16, 64
N_CORES = 8
TPC = T // 2  # tokens per core = 512

_built = None


def _build():
    """Build + compile the 8-core SPMD Bass program once."""
    global _built
    if _built is not None:
        return _built

    import concourse.mybir as mybir
    import concourse.tile as tile
    from concourse import bacc

    f32 = mybir.dt.float32
    bf16 = mybir.dt.bfloat16
    GROUPS = [[0, 1], [2, 3], [4, 5], [6, 7]]

    nc = bacc.Bacc("TRN2", target_bir_lowering=False, debug=False, num_devices=N_CORES)
    # x pre-swizzled: [128 part, e*512 + tok]
    xd = nc.dram_tensor("xd", [128, 4096], bf16, kind="ExternalInput").ap()
    # kv weights: 4 stream groups (k0, v0, k1, v1), each [128 part, e*512 + kvf]
    wkvd = nc.dram_tensor("wkvd", [4 * 128, 8 * 512], bf16, kind="ExternalInput").ap()
    # q weights: 2 column-half groups, each [128 part, e*512 + qf]
    wqd = nc.dram_tensor("wqd", [2 * 128, 8 * 512], bf16, kind="ExternalInput").ap()
    # proj weights, oc-major: [128 part, oc*4096 + f*512 + c]
    wpd = nc.dram_tensor("wpd", [128, 8 * 1024], bf16, kind="ExternalInput").ap()
    out = nc.dram_tensor("out", [TPC, E], f32, kind="ExternalOutput").ap()

    evict_i = [0]

    def evict(dst, src):
        # spread PSUM->SBUF eviction copies across DVE and ACT
        if evict_i[0] % 2 == 0:
            nc.vector.tensor_copy(dst, src)
        else:
            nc.scalar.copy(dst, src)
        evict_i[0] += 1

    with tile.TileContext(nc) as tc:
        with (
            tc.tile_pool(name="xp", bufs=1) as xp,
            tc.tile_pool(name="wkvp", bufs=1) as wkvp,
            tc.tile_pool(name="kvp", bufs=1) as kvp,
            tc.tile_pool(name="wqp", bufs=1) as wqp,
            tc.tile_pool(name="wpp", bufs=1) as wpp,
            tc.tile_pool(name="qp", bufs=1) as qp,
            tc.tile_pool(name="mres", bufs=1) as mres,
            tc.tile_pool(name="op", bufs=2) as op,
            tc.tile_pool(name="warm", bufs=1) as warmp,
            tc.tile_pool(name="dram", bufs=1, space="DRAM") as dram,
            tc.tile_pool(name="psA", bufs=6, space="PSUM") as psA,
            tc.tile_pool(name="psM", bufs=2, space="PSUM") as psM,
        ):
            # ---- t=0: PE p-state warmup while the first DMAs land ----
            warm = warmp.tile([128, 512], bf16, tag="warm")
            nc.gpsimd.memset(warm[:].bitcast(f32), 0.0)
            Mbd = mres.tile([128, 1024], bf16, tag="Mbd")
            nc.gpsimd.memset(Mbd[:].bitcast(f32), 0.0)
            psw = psM.tile([128, 512], f32, tag="mp", name="warm_ps")
            for _ in range(12):
                nc.tensor.matmul(psw[:], warm[:, 0:128], warm[:],
                                 start=True, stop=True)

            # ---- phase-1 input DMAs (6MB; wp deferred to the scalar queue) ----
            xsb = xp.tile([128, 4096], bf16, tag="x")  # col = e*512 + tok
            KV_SLOT = [0, 2, 1, 3]  # stream order k0, v0, k1, v1 -> kvsb col slot
            wkv = [wkvp.tile([128, 4096], bf16, tag=f"wkv{s}", name=f"wkv{s}")
                   for s in range(4)]
            wq = [wqp.tile([128, 4096], bf16, tag=f"wq{h}", name=f"wq{h}")
                  for h in range(2)]
            wp = wpp.tile([128, 8192], bf16, tag="wp")

            nc.sync.dma_start(wkv[0][:], wkvd[0:128, :])
            nc.sync.dma_start(xsb[:], xd[:, :])
            for s in range(1, 4):
                nc.sync.dma_start(wkv[s][:], wkvd[128 * s:128 * (s + 1), :])
            for h in range(2):
                nc.sync.dma_start(wq[h][:], wqd[128 * h:128 * (h + 1), :])

            # kvsb[tt]: [128 tok, 2048] cols = [k(1024) | v(1024)] grouped feats
            kvsb = [kvp.tile([128, 2048], bf16, tag=f"kv{tt}", name=f"kv{tt}")
                    for tt in range(4)]

            def kv_quarter(s):
                slot = KV_SLOT[s]
                for tt in range(4):
                    ps = psA.tile([128, 512], f32, tag="big")
                    for e in range(8):
                        nc.tensor.matmul(
                            ps[:],
                            xsb[:, 512 * e + 128 * tt:512 * e + 128 * (tt + 1)],
                            wkv[s][:, 512 * e:512 * (e + 1)],
                            start=(e == 0), stop=(e == 7),
                        )
                    evict(kvsb[tt][:, 512 * slot:512 * (slot + 1)], ps[:])

            def m_half(g):
                # M blocks 4g..4g+3 (2 heads per 128-block, diagonal 64x64s),
                # AllReduce(add) over the batch's core pair
                mp = psM.tile([128, 512], f32, tag="mp", name=f"mp{g}")
                for j in range(4):
                    blk = 4 * g + j
                    for tt in range(4):
                        nc.tensor.matmul(
                            mp[:, 128 * j:128 * (j + 1)],
                            kvsb[tt][:, 128 * blk:128 * (blk + 1)],
                            kvsb[tt][:, 1024 + 128 * blk:1024 + 128 * (blk + 1)],
                            start=(tt == 0), stop=(tt == 3),
                        )
                # extract the 8 diagonal 64x64 blocks -> Msb [128, 256] bf16
                Msb = mres.tile([128, 256], bf16, tag=f"Msb{g}", name=f"Msb{g}")
                mpv = mp[:].rearrange("p (j c) -> p j c", j=4)
                msv = Msb[:].rearrange("p (j c) -> p j c", j=4)
                nc.vector.tensor_copy(msv[0:64], mpv[0:64, :, 0:64])
                nc.scalar.copy(msv[64:128], mpv[64:128, :, 64:128])
                # deferred wp half rides the scalar queue here, during the
                # collective's op window (issue only, doesn't block the queue)
                nc.scalar.dma_start(wp[:, 4096 * g:4096 * (g + 1)],
                                    wpd[:, 4096 * g:4096 * (g + 1)])
                bin_ = dram.tile([128, 256], bf16, name=f"bin{g}")
                bo = dram.tile([128, 256], bf16, name=f"bo{g}")
                nc.gpsimd.dma_start(bin_[:], Msb[:])
                nc.gpsimd.collective_compute(
                    "AllReduce", mybir.AluOpType.add, replica_groups=GROUPS,
                    ins=[bin_.opt()], outs=[bo.opt()],
                )
                # summed result lands straight in Mbd's diagonal spots
                mbv = Mbd[:, 512 * g:512 * (g + 1)].rearrange("p (j c) -> p j c", j=4)
                bov = bo[:].rearrange("p (j c) -> p j c", j=4)
                nc.sync.dma_start(mbv[0:64, :, 0:64], bov[0:64])
                nc.sync.dma_start(mbv[64:128, :, 64:128], bov[64:128])

            # ---- kv + M + allreduces, pipelined in halves ----
            kv_quarter(0)      # k0
            kv_quarter(1)      # v0
            m_half(0)
            kv_quarter(2)      # k1
            kv_quarter(3)      # v1
            m_half(1)

            # ---- q (feature-major, [128 qf, 512 tok] per block), overlaps CC ----
            qsb = [qp.tile([128, TPC], bf16, tag=f"q{f}", name=f"q{f}")
                   for f in range(8)]
            for fq in range(8):
                wqh = wq[fq // 4]
                l = fq % 4
                ps = psA.tile([128, 512], f32, tag="big")
                for e in range(8):
                    nc.tensor.matmul(
                        ps[:],
                        wqh[:, 512 * e + 128 * l:512 * e + 128 * (l + 1)],
                        xsb[:, 512 * e:512 * (e + 1)],
                        start=(e == 0), stop=(e == 7),
                    )
                evict(qsb[fq][:], ps[:])

            # ---- att blk = Mbd_blk.T @ q_blk, in-place into the q tiles.
            # Blocks 0-3 need only AllReduce#1; 4-7 wait on AllReduce#2, so
            # the out projection is interleaved to hide AR2's latency. ----
            def att(blk):
                ps = psM.tile([128, 512], f32, tag="mp", name=f"att{blk}")
                nc.tensor.matmul(ps[:], Mbd[:, 128 * blk:128 * (blk + 1)],
                                 qsb[blk][:], start=True, stop=True)
                evict(qsb[blk][:], ps[:])
            attsb = qsb

            # PE filler while waiting for AllReduce#1 (keeps the p-state up;
            # ret0 lands well after these, so they never delay real work)
            dum = psA.tile([128, 512], f32, tag="big", name="dum")
            for _ in range(8):
                nc.tensor.matmul(dum[:], warm[:, 0:128], warm[:],
                                 start=True, stop=True)

            for blk in range(4):
                att(blk)

            # open 6 PSUM groups with their f=0..3 partials (att blocks 0-3
            # only), finish f=4..7 once AR2 lands: (oc,tt) = the 4 oc0 rows
            # plus oc1 rows tt0/tt1
            held = []
            for oc, tt in [(0, 0), (0, 1), (0, 2), (0, 3), (1, 0), (1, 1)]:
                ps = psA.tile([128, 512], f32, tag="big", name=f"o{oc}_{tt}")
                for f in range(4):
                    nc.tensor.matmul(
                        ps[:],
                        attsb[f][:, 128 * tt:128 * (tt + 1)],
                        wp[:, 4096 * oc + 512 * f:4096 * oc + 512 * (f + 1)],
                        start=(f == 0), stop=False,
                    )
                held.append((oc, tt, ps))

            for blk in range(4, 8):
                att(blk)

            for oc, tt, ps in held:
                for f in range(4, 8):
                    nc.tensor.matmul(
                        ps[:],
                        attsb[f][:, 128 * tt:128 * (tt + 1)],
                        wp[:, 4096 * oc + 512 * f:4096 * oc + 512 * (f + 1)],
                        start=False, stop=(f == 7),
                    )
                ot = op.tile([128, 512], f32, tag="osb")
                evict(ot[:], ps[:])
                nc.gpsimd.dma_start(
                    out[128 * tt:128 * (tt + 1), 512 * oc:512 * (oc + 1)], ot[:])

            # remaining oc1 rows: everything available, straight accumulation
            for tt in (2, 3):
                ps = psA.tile([128, 512], f32, tag="big", name=f"o1_{tt}")
                for f in range(8):
                    nc.tensor.matmul(
                        ps[:],
                        attsb[f][:, 128 * tt:128 * (tt + 1)],
                        wp[:, 4096 + 512 * f:4096 + 512 * (f + 1)],
                        start=(f == 0), stop=(f == 7),
                    )
                ot = op.tile([128, 512], f32, tag="osb")
                evict(ot[:], ps[:])
                nc.gpsimd.dma_start(out[128 * tt:128 * (tt + 1), 512:1024], ot[:])

    nc.compile()
    _built = nc
    return nc


LAST_RESULTS = None  # BassKernelResults of the most recent kernel() call


def _swz(a: np.ndarray) -> np.ndarray:
    """[1024, C] -> [128, 8*C]: row e*128+p -> partition p, cols e*C..e*C+C."""
    C = a.shape[1]
    return np.ascontiguousarray(
        a.reshape(8, 128, C).transpose(1, 0, 2).reshape(128, 8 * C))


def kernel(x: np.ndarray, W_qkv: np.ndarray, W_proj: np.ndarray) -> np.ndarray:
    global LAST_RESULTS
    from ml_dtypes import bfloat16
    from concourse import bass_utils

    nc = _build()

    x = np.ascontiguousarray(x, dtype=np.float32)
    W_qkv = np.ascontiguousarray(W_qkv, dtype=np.float32)
    W_proj = np.ascontiguousarray(W_proj, dtype=np.float32)

    # head-grouping permutation: grouped feature h*64+j <- original row j*16+h
    perm = np.arange(E).reshape(HD, NH).T.ravel()
    Wq_g = W_qkv[perm].astype(bfloat16)
    Wk_g = (W_qkv[E + perm] * np.float32(HD ** -0.5)).astype(bfloat16)  # exact 1/8
    Wv_g = W_qkv[2 * E + perm].astype(bfloat16)
    Wp_g = W_proj.astype(bfloat16)  # att concat order == grouped order already

    # kv stream groups k0, v0, k1, v1: each [512 kvf, 1024 xf] -> swz([1024, 512])
    kv_groups = [Wk_g[0:512], Wv_g[0:512], Wk_g[512:1024], Wv_g[512:1024]]
    wkvd_np = np.concatenate([_swz(np.ascontiguousarray(g.T)) for g in kv_groups], 0)
    wqd_np = np.concatenate(
        [_swz(np.ascontiguousarray(Wq_g[512 * h:512 * (h + 1)].T)) for h in range(2)], 0)
    # wp oc-major: [128 p(af in f), oc*4096 + f*512 + c], wp[p, ...] = Wp[o, af]
    wpT = np.ascontiguousarray(Wp_g.T)           # [1024 af, 1024 o]
    w = wpT.reshape(8, 128, 2, 512)              # [f, p, oc, c]
    wpd_np = np.ascontiguousarray(
        w.transpose(1, 2, 0, 3).reshape(128, 8192))  # [p, oc, f, c]

    in_maps = []
    for c in range(N_CORES):
        b, half = c // 2, c % 2
        xd_c = _swz(np.ascontiguousarray(
            x[b, half * TPC:(half + 1) * TPC, :].T.astype(bfloat16)))
        in_maps.append({"xd": xd_c, "wkvd": wkvd_np, "wqd": wqd_np, "wpd": wpd_np})

    import os as _os
    _tc = _os.environ.get("KERNEL_TRACE_CORES")
    _kw = {"trace_cores": [int(v) for v in _tc.split(",")]} if _tc else {}
    res = bass_utils.run_bass_kernel_spmd(nc, in_maps, core_ids=list(range(N_CORES)), **_kw)
    LAST_RESULTS = res

    out = np.empty((B, T, E), dtype=np.float32)
    for c in range(N_CORES):
        b, half = c // 2, c % 2
        out[b, half * TPC:(half + 1) * TPC, :] = res.results[c]["out"]
    return out
